# revision 69
# baseline (speedup 1.0000x reference)
"""Trainium2 Bass kernel for nn_Attention_34840774705279 (sparse/deformable attention).

Math (matches reference.py):
  v   = x @ v_w.T                  -> per-head maps [B*NH, H, W, HD]
  off = off_b (off_w == 0)         -> constant integer offsets (dx, dy) = p*(ux, uy)
  w   = softmax_p(x @ aw_w.T)
  out[i,j] = sum_p w_p[i,j] * v[i+dy_p, j+dx_p]   (zero outside the map)
  y   = out @ proj_w.T

Sharding (8 cores, uniform SPMD): core d -> batch b = d//2, row-half r0 =
64*(d%2); each core computes all 8 heads for its 64 output rows using a 4-row
halo of v rows (host zero-pads), so no cross-core traffic; host concatenates.

v2 design (all bf16 data path, f32 PSUM accumulation):
  A. pixel-major projection: per image row r, a_ps[j, 288] = x_row^T @ [v|aw]
     (2 matmuls, contraction 256); single-row PSUM tiles on a 4-deep ring so
     the evacuation round-trip hides under PE work; evac copies (alternating
     ScalarE/DVE) land in VL[j, 288ch, 72slots] with the row-slot innermost
     so the DVE weighting op later hits its 2-byte fast path.
  B. softmax over points: one exp (ScalarE), adds + reciprocal (DVE, f32),
     one batched normalize multiply (DVE bf16 2x mode).
  C. weight-then-shift: heads are host-permuted into [+dx trio, -dx trio,
     dx=0 duo] so the column-shifted weights E' = S_-dx^T E are computed with
     ONE matmul per dx value (3 heads batched); DVE/Pool multiply the V window
     by E' (bf16 2x on DVE; Pool takes every 4th op - it cannot touch PSUM so
     SBUF-only weighting is the one job it can absorb); 0/1 shift matrices
     S_dx matmul-accumulate the 4 points in PSUM (the p-sum rides the PSUM
     accumulation for free).
  D. output projection: PE transposes OUT rows to channel-major (bf16 PSUM),
     y^T = proj^T @ OUT^T; y leaves as bf16, host casts + transposes.
     Software-pipelined: group g+1's transposes are emitted before group g's
     proj matmuls so PE never stalls on the ot evacuation copy.
  Emission interleaves A's tail under C0, D0 under B1/ep1/C1, and splits the
  pure-halo x rows (68-71) out of phase A to fill the C0-duo -> B1 gap.
"""

import os
import sys

import numpy as np

sys.path.insert(0, "/opt/trn_rl_repo")

P = 128
H = W = 128
NH, NP, HD = 8, 4, 32
DIM = 256
N_TOK = H * W
ROWS_OUT = 64          # output rows per core
HALO = 4
ROWS_V = ROWS_OUT + 2 * HALO   # 72 v-row slots per core
TOK_V = ROWS_V * W             # 9216
N_CORES = 8
NCH = DIM + NH * NP    # 288 channels out of the fused projection

_cache = {}


# ---------------------------------------------------------------------------
# geometry: constant offsets -> head permutation + shift matrices
# ---------------------------------------------------------------------------

def _derive_geometry(off_b):
    """For each head h expect offsets (dx, dy) = p*(ux, uy), ux/uy in {-1,0,1},
    integer (bilinear weight ~1). Returns (hord, uys, uxs) with heads permuted
    to [ux=+1 trio, ux=-1 trio, ux=0 duo], or None if the pattern fails."""
    ob = np.asarray(off_b, np.float64).reshape(NH, NP, 2)
    info = []
    for h in range(NH):
        u = None
        for p in range(NP):
            fx, fy = ob[h, p, 0], ob[h, p, 1]
            dx, dy = round(fx), round(fy)
            # must be integer shifts with negligible bilinear remainder
            if abs(fx - dx) > 1e-6 or abs(fy - dy) > 1e-6:
                return None
            if dx % (p + 1) or dy % (p + 1):
                return None
            uu = (dx // (p + 1), dy // (p + 1))
            if abs(uu[0]) > 1 or abs(uu[1]) > 1:
                return None
            if u is None:
                u = uu
            elif u != uu:
                return None
        info.append(u)
    plus = [h for h in range(NH) if info[h][0] == 1]
    minus = [h for h in range(NH) if info[h][0] == -1]
    zero = [h for h in range(NH) if info[h][0] == 0]
    if len(plus) != 3 or len(minus) != 3 or len(zero) != 2:
        return None
    hord = plus + minus + zero
    uys = [info[h][1] for h in hord]
    uxs = [info[h][0] for h in hord]
    return hord, uys, uxs


def _build_smats():
    """smat[k] for k in -4..4: m[j_in, j_out] = 1 at j_in = j_out + k."""
    mats = np.zeros((9, P, P), np.float32)
    for k in range(-4, 5):
        m = mats[k + 4]
        for j_out in range(W):
            j_in = j_out + k
            if 0 <= j_in < W:
                m[j_in, j_out] = 1.0
    return mats


def _np_reference(x, v_w, v_b, aw_w, aw_b, off_w, off_b, proj_w, proj_b, Hh, Ww):
    """Pure-numpy fallback mirroring reference.py (only used off-spec)."""
    B, N, C = x.shape
    v = (x @ v_w.T + v_b).reshape(B, N, NH, HD).transpose(0, 2, 1, 3)
    v = v.reshape(B * NH, Hh, Ww, HD)
    mh, mw = np.meshgrid(np.arange(Hh, dtype=x.dtype), np.arange(Ww, dtype=x.dtype),
                         indexing="ij")
    ref = np.stack([mw, mh], -1).reshape(1, N, 1, 2)
    off = (x @ off_w.T + off_b).reshape(B, N, NH, NP, 2).transpose(0, 2, 1, 3, 4)
    off = off.reshape(B * NH, N, NP, 2)
    grid = ref + off
    w = (x @ aw_w.T + aw_b).reshape(B, N, NH, NP).transpose(0, 2, 1, 3)
    w = w.reshape(B * NH, N, NP)
    w = np.exp(w - w.max(-1, keepdims=True))
    w = w / w.sum(-1, keepdims=True)
    G = B * NH
    vf = v.reshape(G, Hh * Ww, HD)
    gx, gy = grid[..., 0], grid[..., 1]
    x0 = np.floor(gx); y0 = np.floor(gy)
    wx1 = gx - x0; wx0 = 1.0 - wx1
    wy1 = gy - y0; wy0 = 1.0 - wy1
    x0i = x0.astype(np.int64); y0i = y0.astype(np.int64)

    def gather(xi, yi):
        valid = (xi >= 0) & (xi < Ww) & (yi >= 0) & (yi < Hh)
        idx = (np.clip(yi, 0, Hh - 1) * Ww + np.clip(xi, 0, Ww - 1))
        g = np.take_along_axis(vf, idx.reshape(G, -1, 1), axis=1)
        return g.reshape(*xi.shape, HD) * valid[..., None]

    samp = ((wy0 * wx0)[..., None] * gather(x0i, y0i)
            + (wy0 * wx1)[..., None] * gather(x0i + 1, y0i)
            + (wy1 * wx0)[..., None] * gather(x0i, y0i + 1)
            + (wy1 * wx1)[..., None] * gather(x0i + 1, y0i + 1))
    out = np.einsum("gnpd,gnp->gnd", samp, w)
    out = out.reshape(B, NH, N, HD).transpose(0, 2, 1, 3).reshape(B, N, C)
    return (out @ proj_w.T + proj_b).astype(np.float32)


# ---------------------------------------------------------------------------
# device program
# ---------------------------------------------------------------------------

def _build_program(uys):
    import concourse.bass as bass
    import concourse.mybir as mybir
    import concourse.tile as tile
    from concourse import bacc

    dt = mybir.dt
    f32 = dt.float32
    bf16 = dt.bfloat16

    # engine placement knobs. PSUM evacuations may only use scalar/vector
    # (GPSIMD cannot access PSUM on HW); pool takes SBUF-only weighting ops.
    EV_A = os.environ.get("EV_A", "sv")        # rotation for A evacs
    EV_OUT = os.environ.get("EV_OUT", "s")     # rotation for out evacs
    EV_OT0 = os.environ.get("EV_OT0", "s")     # D0 ot evacs
    EV_OT1 = os.environ.get("EV_OT1", "v")     # D1 ot evacs
    EV_Y0 = os.environ.get("EV_Y0", "sv")      # D0 y evacs (mc0, mc1)
    EV_Y1 = os.environ.get("EV_Y1", "ss")      # D1 y evacs
    WEIGHT_ROT = os.environ.get("WEIGHT_ROT", "vvvp")  # weighting ops rotation
    ENG = {"s": "scalar", "v": "vector", "p": "pool"}

    nc = bacc.Bacc("TRN2", target_bir_lowering=False, debug=False,
                   num_devices=N_CORES)

    _rotc = {}

    def evac(which):
        if len(which) > 1:  # rotation string like "sv"
            i = _rotc[which] = (_rotc.get(which, -1) + 1) % len(which)
            which = which[i]
        return {"s": nc.scalar.copy, "v": nc.vector.tensor_copy,
                "scalar": nc.scalar.copy, "vector": nc.vector.tensor_copy,
                "pool": nc.gpsimd.tensor_copy}[which]

    def tt_eng(rot):
        i = _rotc[rot] = (_rotc.get(rot, -1) + 1) % len(rot)
        return {"v": nc.vector, "p": nc.gpsimd, "s": None}[rot[i]]

    INTERLEAVE_C = os.environ.get("INTERLEAVE_C", "0") == "1"

    # ---- DRAM I/O ----
    xt_d = nc.dram_tensor("xt_dev", [DIM, TOK_V], bf16, kind="ExternalInput")
    wb_d = nc.dram_tensor("wb_cat", [2, P, NCH], bf16, kind="ExternalInput")
    s_d = nc.dram_tensor("s_mats", [9, P, P], bf16, kind="ExternalInput")
    pj_d = nc.dram_tensor("proj_t", [2, 2, P, P], bf16, kind="ExternalInput")
    id_d = nc.dram_tensor("ident", [P, P], bf16, kind="ExternalInput")
    y_d = [nc.dram_tensor(f"y{mc}", [P, ROWS_OUT * W], bf16,
                          kind="ExternalOutput") for mc in range(2)]

    NG = 9  # x DMA groups of 8 rows

    with tile.TileContext(nc) as tc:
        with (
            tc.tile_pool(name="const", bufs=1) as cpool,
            tc.tile_pool(name="big", bufs=1) as bigpool,
            tc.tile_pool(name="stA", bufs=2) as stA,
            tc.tile_pool(name="stB", bufs=2) as stB,
            tc.tile_pool(name="stM", bufs=1) as stM,
            tc.tile_pool(name="stD", bufs=2) as stD,
            tc.tile_pool(name="psA", bufs=2, space="PSUM") as psA,
            tc.tile_pool(name="psC", bufs=2, space="PSUM") as psC,
        ):
            # ---- constants (only wb gates phase A; rest loaded later).
            # wb is split per contraction half so the first matmul only
            # waits for its own half's DMA. ----
            wb_sb = cpool.tile([P, 2, NCH], bf16, tag="wb")
            wb_kc = [wb_sb[:, 0, :], wb_sb[:, 1, :]]

            def load_wb():
                nc.sync.dma_start(wb_sb[:],
                                  wb_d.rearrange("kc k f -> k kc f"))
            s_sb = cpool.tile([P, 9, P], bf16, tag="smats")
            pj_sb = cpool.tile([P, 2, 2, P], bf16, tag="proj")
            id_sb = cpool.tile([P, P], bf16, tag="ident")

            def load_late_consts():
                nc.sync.dma_start(s_sb[:], s_d.rearrange("s k f -> k s f"))
                nc.sync.dma_start(pj_sb[:],
                                  pj_d.rearrange("kc m k f -> k kc m f"))
                nc.sync.dma_start(id_sb[:], id_d[:])

            def sf(dx):  # forward shift matrix index
                return dx + 4

            def sb(dx):  # backward (S_-dx) index
                return -dx + 4

            # ---- persistent tiles ----
            # VL: [j, 288 ch, 72 row-slots]; ch = h'*32+d for v, 256+h'*4+p logits
            vl = bigpool.tile([P, NCH, ROWS_V], bf16, tag="VL")
            es = [bigpool.tile([P, NH * NP, 32], bf16, tag="E", name=f"e{h}")
                  for h in range(2)]
            eps = [bigpool.tile([P, 8, 3, 32], bf16, tag="EP", name=f"ep{h}")
                   for h in range(2)]
            outs = [bigpool.tile([P, DIM, 32], bf16, tag="OUT", name=f"out{h}")
                    for h in range(2)]

            def phase_a_dma(row0, nrows, mid=None):
                xt_g = [stA.tile([P, 2048], bf16, tag=f"xt{kc}", bufs=3,
                                 name=f"xtg{kc}") for kc in range(2)]
                for kc in range(2):
                    nc.sync.dma_start(
                        xt_g[kc][:, :P * nrows],
                        xt_d[P * kc:P * kc + P,
                             P * row0:P * (row0 + nrows)])
                    if kc == 0 and mid is not None:
                        mid()
                return xt_g

            def phase_a(row0, nrows, early, after_dma=None, xt_g=None):
                """x rows row0..row0+nrows: fused v+logit projection.

                Single-row PSUM tiles with a 4-deep ring: the evac round-trip
                (copy + 2 sem hops ~= 850ns) hides under 4 rows of PE work."""
                if xt_g is None:
                    xt_g = phase_a_dma(row0, nrows, mid=after_dma)
                for rl in range(nrows):
                    a_ps = psA.tile([P, 512], f32, tag="a", bufs=4)
                    for kc in range(2):
                        nc.tensor.matmul(
                            a_ps[:, :NCH],
                            xt_g[kc][:, P * rl:P * rl + P],
                            wb_kc[kc][:], start=(kc == 0),
                            stop=(kc == 1))
                    s0 = row0 + rl
                    evac(EV_A)(vl[:, :, s0], a_ps[:, :NCH])

            def phase_b(half, eng=None):
                """exp + softmax over the 4 points, all heads, 32 rows."""
                eng = eng or nc.vector
                rr = 32 * half
                e_sb = es[half]
                nc.scalar.activation(
                    e_sb[:], vl[:, DIM:, HALO + rr:HALO + rr + 32],
                    mybir.ActivationFunctionType.Exp)
                ev = e_sb[:].rearrange("j (h p) i -> j h p i", p=NP)
                z0 = stB.tile([P, NH, 32], f32, tag="z0")
                z1 = stB.tile([P, NH, 32], f32, tag="z1")
                zr = stB.tile([P, NH, 32], f32, tag="zr")
                zb = stB.tile([P, NH, 32], bf16, tag="zb")
                eng.tensor_tensor(z0[:], ev[:, :, 0, :], ev[:, :, 1, :],
                                  op=mybir.AluOpType.add)
                eng.tensor_tensor(z1[:], ev[:, :, 2, :], ev[:, :, 3, :],
                                  op=mybir.AluOpType.add)
                eng.tensor_tensor(z0[:], z0[:], z1[:],
                                  op=mybir.AluOpType.add)
                nc.vector.reciprocal(zr[:], z0[:])
                nc.vector.tensor_copy(zb[:], zr[:])
                eng.tensor_tensor(
                    ev[:], ev[:],
                    zb[:].unsqueeze(2).broadcast_to([P, NH, NP, 32]),
                    op=mybir.AluOpType.mult)

            def phase_ep(half):
                """column-shifted weights E' = S_-dx^T E, batched 3 heads/dx."""
                e_sb = es[half]
                for rnd in range(2):           # rnd 0: dx=+1..+4, 1: dx=-1..-4
                    sgn = 1 if rnd == 0 else -1
                    hoff = 0 if rnd == 0 else 3
                    ep_ps = psC.tile([P, 4, 128], f32, tag="ep", bufs=1)
                    for k in range(1, 5):
                        dx = sgn * k
                        # heads hoff..hoff+2, point k-1: slots h'*4+(k-1)
                        rhs = e_sb[:].rearrange("j (h p) i -> j h p i", p=NP)[
                            :, hoff:hoff + 3, k - 1, :]
                        nc.tensor.matmul(ep_ps[:, k - 1, :96],
                                         s_sb[:, sb(dx), :], rhs,
                                         start=True, stop=True)
                    evac(os.environ.get("EV_EP", "s"))(
                        eps[half][:, 4 * rnd:4 * rnd + 4],
                        ep_ps[:, :, :96].rearrange("j k (t i) -> j k t i", t=3))

            def _weight_slice(half, hp, p):
                """weight row [P, 32] for (permuted head hp, point p)."""
                if hp < 6:
                    rnd = 0 if hp < 3 else 1
                    return eps[half][:, 4 * rnd + p, hp % 3, :]
                return es[half][:, NP * hp + p, :]

            WEIGHT_BATCH = os.environ.get("WEIGHT_BATCH", "0") == "1"

            def _c_weight(half, m_t, t, hp):
                """weighting multiplies for one head into m_t[:, t]."""
                rr = 32 * half
                uy = uys[hp]
                if WEIGHT_BATCH:
                    # one op per head: overlapping-window AP over p
                    s1 = rr + HALO + uy
                    v_ap = vl[:, HD * hp:HD * hp + HD, s1:s1 + 32] \
                        .unsqueeze(1).broadcast_to([P, NP, HD, 32])
                    v_ap.ap[1] = [uy, NP]
                    w_ap = _weight_slice(half, hp, 0) \
                        .unsqueeze(1).unsqueeze(1) \
                        .broadcast_to([P, NP, HD, 32])
                    w_ap.ap[1] = [96, NP] if hp < 6 else [32, NP]
                    tt_eng(WEIGHT_ROT).tensor_tensor(
                        m_t[:, t], v_ap, w_ap, op=mybir.AluOpType.mult)
                    return
                for p in range(NP):
                    s0 = rr + HALO + uy * (p + 1)
                    tt_eng(WEIGHT_ROT).tensor_tensor(
                        m_t[:, t, p],
                        vl[:, HD * hp:HD * hp + HD, s0:s0 + 32],
                        _weight_slice(half, hp, p)
                        .unsqueeze(1).broadcast_to([P, HD, 32]),
                        op=mybir.AluOpType.mult)

            def _c_shift(half, m_t, t, hp, out_eng):
                """shift-accumulate + out evac for one head."""
                dxu = 1 if hp < 3 else (-1 if hp < 6 else 0)
                o_ps = [psC.tile([P, 512], f32, tag="o", bufs=3,
                                 name=f"o{ch}") for ch in range(2)]
                for p in range(NP):
                    dx = dxu * (p + 1)
                    mv = m_t[:, t, p].rearrange("j d i -> j (d i)")
                    for ch in range(2):
                        nc.tensor.matmul(
                            o_ps[ch][:], s_sb[:, sf(dx), :],
                            mv[:, 512 * ch:512 * ch + 512],
                            start=(p == 0), stop=(p == NP - 1))
                for ch in range(2):
                    evac(out_eng or EV_OUT)(
                        outs[half][:, HD * hp + 16 * ch:HD * hp + 16 * ch + 16, :],
                        o_ps[ch][:].rearrange("j (d i) -> j d i", d=16))

            def phase_c_heads(half, heads, mtag, out_eng=None):
                """weighting + shift-accumulate + evac for a group of heads."""
                m_t = stM.tile([P, 3, NP, 32, 32], bf16,
                               tag=mtag, bufs=1, name=mtag)
                if INTERLEAVE_C:
                    for t, hp in enumerate(heads):
                        _c_weight(half, m_t, t, hp)
                        _c_shift(half, m_t, t, hp, out_eng)
                else:
                    for t, hp in enumerate(heads):
                        _c_weight(half, m_t, t, hp)
                    for t, hp in enumerate(heads):
                        _c_shift(half, m_t, t, hp, out_eng)

            # ---- phase D: software-pipelined (trans(g+1) emitted before
            # proj(g) so PE never stalls on the ot evacuation copy) ----
            d_state = {}

            def d_trans(half, gl, ot_eng=None):
                ot_eng = ot_eng or (EV_OT0 if half == 0 else EV_OT1)
                """transposes of out-row group gl to channel-major."""
                i0 = 4 * gl
                ot_ps = psA.tile([P, 8, P], bf16, tag="a",
                                 name="otp", bufs=4)
                for kc in range(2):
                    for il in range(4):
                        nc.tensor.transpose(
                            ot_ps[:, 4 * kc + il, :],
                            outs[half][:, P * kc:P * kc + P, i0 + il],
                            id_sb[:])
                ot_sb = stD.tile([P, 2, 512], bf16, tag="ot", bufs=2)
                evac(ot_eng)(
                    ot_sb[:].rearrange("j kc f -> j (kc f)"),
                    ot_ps[:].rearrange("j a b -> j (a b)"))
                d_state[(half, gl)] = ot_sb

            def d_proj(half, gl, y_engs=None):
                """output projection + y evac (+DMA every 4th group)."""
                y_engs = y_engs or (EV_Y0 if half == 0 else EV_Y1)
                if half == 1 and gl >= 6:
                    y_engs = "sv"   # parallel evac on the critical tail
                g = 8 * half + gl
                ot_sb = d_state.pop((half, gl))
                y_ps = [psA.tile([P, 512], f32, tag="a",
                                 name=f"yps{mc}", bufs=4) for mc in range(2)]
                for mc in range(2):
                    for kc in range(2):
                        nc.tensor.matmul(
                            y_ps[mc][:], pj_sb[:, kc, mc, :],
                            ot_sb[:, kc, :],
                            start=(kc == 0), stop=(kc == 1))
                q = gl % 2
                ysb = d_state.get(("ysb", half, gl // 2))
                if ysb is None:
                    ysb = stD.tile([P, 2, 2, 512], bf16, tag="y", bufs=3)
                    d_state[("ysb", half, gl // 2)] = ysb
                for mc in range(2):
                    evac(y_engs[mc])(ysb[:, mc, q, :], y_ps[mc][:])
                if q == 1:
                    g0 = 512 * (g - 1)
                    for mc in range(2):
                        nc.sync.dma_start(
                            y_d[mc][:, g0:g0 + 1024], ysb[:, mc, :, :])

            # ---- emission ----
            phase_a(0, 8, True, after_dma=load_wb)
            phase_a(8, 8, True)
            phase_a(16, 16, True)
            load_late_consts()
            phase_a(32, 16, True)
            phase_b(0)           # no PE work; overlaps A tail
            phase_a(48, 16, False)
            phase_ep(0)
            phase_c_heads(0, [0, 1, 2], "m0")
            phase_a(64, 4, False)
            phase_c_heads(0, [3, 4, 5], "m1")
            xt68 = phase_a_dma(68, 4)   # prefetch halo rows
            phase_c_heads(0, [6, 7], "m0", "sv")
            phase_a(68, 4, False, xt_g=xt68)  # fills the duo->B1 gap
            phase_b(1)
            d_trans(0, 0)   # fill the B1 latency with D0 groups
            d_trans(0, 1); d_proj(0, 0)
            d_trans(0, 2); d_proj(0, 1)
            d_trans(0, 3); d_proj(0, 2)
            phase_ep(1)
            d_trans(0, 4); d_proj(0, 3)
            phase_c_heads(1, [0, 1, 2], "m1")
            d_trans(0, 5); d_proj(0, 4)
            phase_c_heads(1, [3, 4, 5], "m0")
            d_trans(0, 6); d_proj(0, 5)
            phase_c_heads(1, [6, 7], "m1", "sv")
            d_trans(0, 7); d_proj(0, 6)
            d_proj(0, 7)
            d_trans(1, 0)
            for gl in range(1, 8):
                d_trans(1, gl); d_proj(1, gl - 1)
            d_proj(1, 7)

    nc.compile()
    return nc


# ---------------------------------------------------------------------------
# host wrapper
# ---------------------------------------------------------------------------

def kernel(x, v_w, v_b, aw_w, aw_b, off_w, off_b, proj_w, proj_b, H=128, W=128,
           **_unused):
    import ml_dtypes
    bf = ml_dtypes.bfloat16

    x = np.ascontiguousarray(np.asarray(x, np.float32))
    v_w = np.asarray(v_w, np.float32); v_b = np.asarray(v_b, np.float32)
    aw_w = np.asarray(aw_w, np.float32); aw_b = np.asarray(aw_b, np.float32)
    off_w = np.asarray(off_w, np.float32); off_b = np.asarray(off_b, np.float32)
    proj_w = np.asarray(proj_w, np.float32); proj_b = np.asarray(proj_b, np.float32)

    geom = _derive_geometry(off_b)
    if (np.any(off_w != 0.0) or int(H) != 128 or int(W) != 128 or geom is None
            or np.any(v_b) or np.any(aw_b) or np.any(proj_b)):
        return _np_reference(x, v_w, v_b, aw_w, aw_b, off_w, off_b,
                             proj_w, proj_b, int(H), int(W))
    hord, uys, _uxs = geom

    key = ("prog2", tuple(uys))
    if key not in _cache:
        _cache[key] = _build_program(uys)
    nc = _cache[key]

    B = x.shape[0]
    # ---- host prep (shared across cores) ----
    # channel permutation: ch' = h'*32+d  <-  orig h*32+d
    vperm = np.concatenate([np.arange(HD) + HD * h for h in hord])
    aperm = np.concatenate([np.arange(NP) + NP * h for h in hord])
    wb_cat = np.empty((2, P, NCH), np.float32)
    v_wp = v_w[vperm]          # [256 out-ch', 256 in]
    aw_wp = aw_w[aperm]        # [32 out-ch', 256 in]
    for kc in range(2):
        wb_cat[kc, :, :DIM] = v_wp[:, P * kc:P * (kc + 1)].T
        wb_cat[kc, :, DIM:] = aw_wp[:, P * kc:P * (kc + 1)].T
    pj_perm = proj_w[:, vperm]  # permute contraction columns
    pj_t = np.empty((2, 2, P, P), np.float32)
    for kc in range(2):
        for mc in range(2):
            pj_t[kc, mc] = pj_perm[P * mc:P * (mc + 1), P * kc:P * (kc + 1)].T
    shared = dict(wb_cat=np.ascontiguousarray(wb_cat).astype(bf),
                  s_mats=np.ascontiguousarray(_build_smats()).astype(bf),
                  proj_t=np.ascontiguousarray(pj_t).astype(bf),
                  ident=np.eye(P, dtype=np.float32).astype(bf))

    xr = x.reshape(B, H, W, DIM)
    in_maps = []
    for d in range(N_CORES):
        b, half = d // 2, d % 2
        r0 = ROWS_OUT * half
        x_dev = np.zeros((ROWS_V, W, DIM), np.float32)
        lo, hi = max(0, r0 - HALO), min(H, r0 + ROWS_OUT + HALO)
        x_dev[lo - (r0 - HALO):hi - (r0 - HALO)] = xr[b, lo:hi]
        m = dict(shared)
        m["xt_dev"] = np.ascontiguousarray(
            x_dev.reshape(TOK_V, DIM).T).astype(bf)
        in_maps.append(m)

    from concourse import bass_utils
    res = bass_utils.run_bass_kernel_spmd(
        nc, in_maps, core_ids=list(range(N_CORES)),
        trace=os.environ.get("KERNEL_TRACE", "0") == "1")
    kernel.last_results = res

    y = np.empty((B, N_TOK, DIM), np.float32)
    for d in range(N_CORES):
        b, half = d // 2, d % 2
        yd = np.concatenate([np.asarray(res.results[d]["y0"]),
                             np.asarray(res.results[d]["y1"])], 0)
        y[b, ROWS_OUT * W * half:ROWS_OUT * W * (half + 1), :] = \
            yd.astype(np.float32).T
    return y
